# revision 15
# baseline (speedup 1.0000x reference)
"""DGAT (edge-featured multi-head GAT layer + FF) Trainium2 Bass kernel.

Strategy (graph/data parallel over 8 NeuronCores):
  - Nodes are partitioned into 8 contiguous destination-node shards; each core
    owns the edges whose dst falls in its shard (host sorts edges by dst).
  - Each core builds a node table T[n] = [h'(64, head-major) | su(8) | sv(8)]
    for ALL nodes (replicated compute; h = x@w_in+b etc. folded into one
    matmul with an augmented-ones row for biases).
  - Edges are packed into "virtual rows" of 8 slots each (vrow = up to 8 edges
    sharing one dst). Chunks of <=112 dst nodes own <=256 vrows (2 vtiles of
    128). Per-edge source features arrive via indirect-DMA gathers of T rows.
  - Scores: se+sv come from one matmul with a combined stationary
    [one-hot(dst) | per-slot edge features] against [sv+b_eff | w_eff diag];
    su comes from the gathered T rows. LeakyReLU+exp on the scalar engine.
  - Weighted messages + softmax denominators are segment-summed over each
    chunk via a one-hot matmul into PSUM, normalized, then the FF block
    (gelu MLP) runs per chunk fully on-chip. Results scatter back by node id.
"""

import math
import os
from contextlib import ExitStack

import numpy as np

import concourse.bass as bass
import concourse.tile as tile
from concourse import bacc, mybir
from concourse.bass_utils import run_bass_kernel_spmd

F32 = mybir.dt.float32
I32 = mybir.dt.int32

# Problem constants (hardcoded per contract).
CFG = dict(
    N=100000,
    E=1600000,
    DIM=64,
    H=8,
    HD=8,
    HIDDEN=256,
    NCORES=8,
    SLOTS=8,          # edge slots per virtual row
    NVT=2,            # vtiles (of 128 vrows) per chunk
    CN_MAX=112,       # max dst nodes per chunk (112 + 16 ef rows = 128)
    TRASH_SU=-80.0,   # su value of the trash table row: exp(lrelu(-80)) ~ 1e-7
)


# --------------------------------------------------------------------------
# Host-side preprocessing
# --------------------------------------------------------------------------

def _fmap(cfg):
    """f' (head-major: H*HD+hd) -> original feature index f = hd*H + H_idx."""
    H, HD = cfg["H"], cfg["HD"]
    fp = np.arange(H * HD)
    Hi, hd = fp // HD, fp % HD
    return (hd * H + Hi).astype(np.int64)


def _prep_weights(cfg, w_in, b_in, w_edge, b_edge, w_att_u, b_att_u, w_att_v,
                  w_att_e, b_att_e, w_ff1, b_ff1, w_ff2, b_ff2):
    H, HD, DIM, SLOTS = cfg["H"], cfg["HD"], cfg["DIM"], cfg["SLOTS"]
    fm = _fmap(cfg)
    w_in_p = w_in[:, fm]                       # columns permuted to head-major
    b_in_p = b_in[fm]
    w_uv = np.concatenate([w_att_u, w_att_v], axis=1)      # [64,16]
    w_uvfold = w_in @ w_uv                                  # [64,16]
    b_su = b_in @ w_att_u + b_att_u
    b_sv = b_in @ w_att_v
    # rows: 0:64 weights, 64 bias (ones row), 65 "is_trash" indicator row
    waug = np.zeros((66, 80), np.float32)
    waug[:64, 0:64] = w_in_p
    waug[:64, 64:80] = w_uvfold
    waug[64, 0:64] = b_in_p
    waug[64, 64:72] = b_su
    waug[64, 72:80] = b_sv
    waug[65, 64:72] = cfg["TRASH_SU"]

    w_eff = w_edge @ w_att_e                                # [2,8]
    b_eff = b_edge @ w_att_e + b_att_e                      # [8]
    # W rows 112:128: row 2*s+j, col s2*8+Hh = (s==s2) * w_eff[j,Hh]
    wsd = np.zeros((16, 64), np.float32)
    for s in range(SLOTS):
        wsd[2 * s, s * 8:(s + 1) * 8] = w_eff[0]
        wsd[2 * s + 1, s * 8:(s + 1) * 8] = w_eff[1]
    beff_bc = np.tile(b_eff[None, :], (128, SLOTS)).astype(np.float32)  # [128,64]

    e8 = np.zeros((8, 64), np.float32)
    for Hh in range(H):
        e8[Hh, Hh * 8:(Hh + 1) * 8] = 1.0

    wff1_p = w_ff1[fm, :]                                   # rows permuted
    bff1_cols = b_ff1.reshape(2, 128).T.astype(np.float32).copy()  # [128,2]
    bff2_bc = np.tile(b_ff2[None, :], (128, 1)).astype(np.float32)  # [128,64]
    return dict(
        waug=waug, wsd=wsd, beff_bc=beff_bc, e8=e8,
        wff1=np.ascontiguousarray(wff1_p, np.float32),
        wff2=np.ascontiguousarray(w_ff2, np.float32),
        bff1_cols=bff1_cols, bff2_bc=bff2_bc,
    )


def _prep_graph(cfg, x, edge_feat, src, dst):
    """Build per-core chunk/vrow arrays. Returns (per_core list, nchunk, aux)."""
    N, E, NCORES = cfg["N"], cfg["E"], cfg["NCORES"]
    SLOTS, NVT, CN_MAX = cfg["SLOTS"], cfg["NVT"], cfg["CN_MAX"]
    VPC = NVT * 128                       # vrows per chunk
    NPC = N // NCORES                     # nodes per core
    N_PAD = ((N + 511) // 512) * 512      # phase-A padded rows
    if N_PAD == N:
        N_PAD += 512                      # need at least one trash column
    TRASH = N                             # trash row index in T (a pad row)

    src = np.asarray(src).astype(np.int64).ravel()
    dst = np.asarray(dst).astype(np.int64).ravel()
    ef = np.asarray(edge_feat, np.float32).reshape(E, 2)

    order = np.argsort(dst, kind="stable")
    dst_s = dst[order]
    src_s = src[order]
    ef_s = ef[order]

    deg = np.bincount(dst_s, minlength=N)
    node_start = np.concatenate([[0], np.cumsum(deg)])     # [N+1]
    nvr = np.maximum(1, (deg + SLOTS - 1) // SLOTS)        # vrows per node
    # edge -> (vrow_within_node, slot)
    erank = np.arange(E) - node_start[dst_s]               # rank within node
    e_vr_in_node = erank // SLOTS
    e_slot = erank % SLOTS

    # Greedy chunking per core: <=CN_MAX nodes and <=VPC vrows per chunk.
    chunks_per_core = []       # list of list of (n0, n1) node ranges (global)
    for c in range(NCORES):
        lo, hi = c * NPC, (c + 1) * NPC
        ch = []
        n0 = lo
        while n0 < hi:
            n1 = n0
            vsum = 0
            while n1 < hi and (n1 - n0) < CN_MAX and vsum + nvr[n1] <= VPC:
                vsum += nvr[n1]
                n1 += 1
            assert n1 > n0, f"node {n0} has nvr {nvr[n0]} > {VPC}"
            ch.append((n0, n1))
            n0 = n1
        chunks_per_core.append(ch)
    nchunk = max(len(ch) for ch in chunks_per_core)

    per_core = []
    for c in range(NCORES):
        lo = c * NPC
        ch = chunks_per_core[c]
        A_gather = np.full((nchunk, 128), TRASH, np.int32)
        A_scatter = np.full((nchunk, 128), NPC, np.int32)
        A_src = np.full((nchunk, 128, NVT * SLOTS), TRASH, np.int32)
        A_dlc = np.full((nchunk, 128, NVT), -1.0, np.float32)
        A_dlr = np.full((nchunk, NVT * 128), -1.0, np.float32)
        A_ef = np.zeros((nchunk, 2 * SLOTS, NVT * 128), np.float32)
        for k, (n0, n1) in enumerate(ch):
            cn = n1 - n0
            nodes = np.arange(n0, n1)
            A_gather[k, :cn] = nodes
            A_scatter[k, :cn] = nodes - lo
            # vrows of this chunk: node d contributes nvr[d] vrows
            vr_nodes = np.repeat(nodes, nvr[n0:n1])        # [<=VPC] global ids
            nv = len(vr_nodes)
            dloc = (vr_nodes - n0).astype(np.float32)
            A_dlr[k, :nv] = dloc
            # vrow index layout: flat v in [0, VPC) -> (vtile v//128, row v%128)
            vt, vp = np.arange(nv) // 128, np.arange(nv) % 128
            A_dlc[k, vp, vt] = dloc
            # edges of this chunk
            e0, e1 = node_start[n0], node_start[n1]
            if e1 > e0:
                ee = np.arange(e0, e1)
                # flat vrow id within chunk
                vr_base = np.concatenate([[0], np.cumsum(nvr[n0:n1])])
                evr = vr_base[dst_s[ee] - n0] + e_vr_in_node[ee]
                es = e_slot[ee]
                evt, evp = evr // 128, evr % 128
                A_src[k, evp, evt * SLOTS + es] = src_s[ee]
                # ef rows: row 2*s+j, col vt*128+vp
                A_ef[k, 2 * es, evt * 128 + evp] = ef_s[ee, 0]
                A_ef[k, 2 * es + 1, evt * 128 + evp] = ef_s[ee, 1]
        per_core.append(dict(
            A_gather=A_gather, A_scatter=A_scatter, A_src=A_src,
            A_dlc=A_dlc, A_dlr=A_dlr, A_ef=A_ef,
        ))

    x = np.asarray(x, np.float32)
    xT_aug = np.zeros((66, N_PAD), np.float32)
    xT_aug[:64, :N] = x.T
    xT_aug[64, :N] = 1.0
    xT_aug[65, N:] = 1.0                  # pad columns become trash rows of T
    aux = dict(N_PAD=N_PAD, NPC=NPC, nchunk=nchunk)
    return per_core, xT_aug, aux


# --------------------------------------------------------------------------
# Device program
# --------------------------------------------------------------------------

def _build_program(cfg, nchunk, n_pad, npc):
    SLOTS, NVT, CN = cfg["SLOTS"], cfg["NVT"], cfg["CN_MAX"]
    TRASH_SU = cfg["TRASH_SU"]
    NVROW = NVT * 128
    NSL = NVT * SLOTS

    nc = bacc.Bacc("TRN2", target_bir_lowering=False, debug=False)

    def din(name, shape, dt=F32):
        return nc.dram_tensor(name, list(shape), dt, kind="ExternalInput").ap()

    xT = din("xT", (66, n_pad))
    waug_d = din("waug", (66, 80))
    wsd_d = din("wsd", (16, 64))
    beff_d = din("beff_bc", (128, 64))
    e8_d = din("e8", (8, 64))
    wff1_d = din("wff1", (64, 256))
    wff2_d = din("wff2", (256, 64))
    bff1_d = din("bff1_cols", (128, 2))
    bff2_d = din("bff2_bc", (128, 64))
    ag_d = din("A_gather", (nchunk, 128), I32)
    as_d = din("A_scatter", (nchunk, 128), I32)
    asrc_d = din("A_src", (nchunk, 128, NSL), I32)
    adlc_d = din("A_dlc", (nchunk, 128, NVT))
    adlr_d = din("A_dlr", (nchunk, NVROW))
    aef_d = din("A_ef", (nchunk, 2 * SLOTS, NVROW))

    T = nc.dram_tensor("Ttab", [n_pad, 80], F32, kind="Internal").ap()
    out_d = nc.dram_tensor("out", [npc + 1, 64], F32,
                           kind="ExternalOutput").ap()

    with ExitStack() as ctx:
        tc = ctx.enter_context(tile.TileContext(nc))
        cpool = ctx.enter_context(tc.tile_pool(name="consts", bufs=1))

        # ---- constants in SBUF
        waug_sb = cpool.tile([66, 80], F32)
        nc.sync.dma_start(out=waug_sb[:], in_=waug_d[:])
        beff_sb = cpool.tile([128, 64], F32)
        nc.sync.dma_start(out=beff_sb[:], in_=beff_d[:])
        e8_sb = cpool.tile([8, 64], F32)
        nc.sync.dma_start(out=e8_sb[:], in_=e8_d[:])
        wff1_sb = cpool.tile([64, 256], F32)
        nc.sync.dma_start(out=wff1_sb[:], in_=wff1_d[:])
        wff2_sb = cpool.tile([128, 128], F32)
        # [256,64] -> [128, 2, 64]: row r=half*128+p  maps to [p, half, :]
        nc.sync.dma_start(
            out=wff2_sb[:].rearrange("p (h c) -> p h c", h=2),
            in_=wff2_d[:].rearrange("(h p) c -> p h c", h=2))
        bff1_sb = cpool.tile([128, 2], F32)
        nc.sync.dma_start(out=bff1_sb[:], in_=bff1_d[:])
        bff2_sb = cpool.tile([128, 64], F32)
        nc.sync.dma_start(out=bff2_sb[:], in_=bff2_d[:])

        ones_sb = cpool.tile([1, 128], F32)
        nc.vector.memset(ones_sb[:], 1.0)
        iotac_i = cpool.tile([128, 1], I32)
        nc.gpsimd.iota(iotac_i[:], pattern=[[0, 1]], base=0,
                       channel_multiplier=1)
        iotac = cpool.tile([128, 1], F32)
        nc.vector.tensor_copy(out=iotac[:], in_=iotac_i[:])
        iotar_i = cpool.tile([128, 128], I32)
        nc.gpsimd.iota(iotar_i[:], pattern=[[1, 128]], base=0,
                       channel_multiplier=0)
        iotar = cpool.tile([128, 128], F32)
        nc.vector.tensor_copy(out=iotar[:], in_=iotar_i[:])

        # persistent combined-stationary W tile: rows 0:112 per-chunk sv+beff,
        # rows 112:128 constant w_eff slot-diagonal
        wc = cpool.tile([128, 64], F32)
        nc.sync.dma_start(out=wc[112:128, :], in_=wsd_d[:])

        # ---- Phase A: node table  T[n] = [h' | su | sv]
        with tc.tile_pool(name="phaseA", bufs=3) as apool, \
                tc.tile_pool(name="phaseA_ps", bufs=2, space="PSUM") as apsum:
            ngroups = n_pad // 512
            for g in range(ngroups):
                xa = apool.tile([66, 512], F32)
                nc.sync.dma_start(out=xa[:], in_=xT[:, g * 512:(g + 1) * 512])
                ps = apsum.tile([128, 320], F32, space="PSUM")
                for j in range(4):
                    nc.tensor.matmul(ps[:, j * 80:(j + 1) * 80],
                                     lhsT=xa[:, j * 128:(j + 1) * 128],
                                     rhs=waug_sb[:], start=True, stop=True)
                ts = apool.tile([128, 320], F32)
                nc.scalar.copy(ts[:], ps[:])
                nc.sync.dma_start(
                    out=T[g * 512:(g + 1) * 512, :].rearrange(
                        "(j p) c -> p j c", p=128),
                    in_=ts[:].rearrange("p (j c) -> p j c", j=4))

        # ---- Phase B: chunks
        bpool = ctx.enter_context(tc.tile_pool(name="chunk", bufs=2))
        gpool = ctx.enter_context(tc.tile_pool(name="gath", bufs=2))
        bpsum = ctx.enter_context(tc.tile_pool(name="chunk_ps", bufs=1,
                                               space="PSUM"))
        for ck in range(nchunk):
            gidx = bpool.tile([128, 1], I32)
            nc.sync.dma_start(out=gidx[:], in_=ag_d[ck, :, None])
            sidx = bpool.tile([128, 1], I32)
            nc.sync.dma_start(out=sidx[:], in_=as_d[ck, :, None])
            srcx = bpool.tile([128, NSL], I32)
            nc.sync.dma_start(out=srcx[:], in_=asrc_d[ck])
            dlc = bpool.tile([128, NVT], F32)
            nc.sync.dma_start(out=dlc[:], in_=adlc_d[ck])
            dlr = bpool.tile([1, NVROW], F32)
            nc.sync.dma_start(out=dlr[:], in_=adlr_d[ck, None, :])

            # gather chunk-node rows (for sv) and build W rows 0:112
            tch = bpool.tile([128, 80], F32)
            nc.gpsimd.indirect_dma_start(
                out=tch[:], out_offset=None, in_=T[:],
                in_offset=bass.IndirectOffsetOnAxis(ap=gidx[:, 0:1], axis=0))
            nc.vector.tensor_tensor(
                out=wc[0:CN, :].rearrange("p (s c) -> p s c", s=SLOTS),
                in0=tch[0:CN, 72:80].unsqueeze(1).to_broadcast(
                    [CN, SLOTS, 8]),
                in1=beff_sb[0:CN, :].rearrange("p (s c) -> p s c", s=SLOTS),
                op=mybir.AluOpType.add)

            # U = [one-hot(dloc) ; ef rows]
            dbc = bpsum.tile([CN, NVROW], F32, space="PSUM")
            nc.tensor.matmul(dbc[:], lhsT=ones_sb[0:1, 0:CN],
                             rhs=dlr[0:1, :], start=True, stop=True)
            u_t = bpool.tile([128, NVROW], F32)
            nc.sync.dma_start(out=u_t[CN:128, :], in_=aef_d[ck])
            nc.vector.tensor_tensor(
                out=u_t[0:CN, :], in0=dbc[:],
                in1=iotac[0:CN, 0:1].to_broadcast([CN, NVROW]),
                op=mybir.AluOpType.is_equal)

            # sv+se+b_eff per (vrow, slot, head)
            sps = bpsum.tile([128, NVT * 64], F32, space="PSUM")
            for v in range(NVT):
                nc.tensor.matmul(sps[:, v * 64:(v + 1) * 64],
                                 lhsT=u_t[:, v * 128:(v + 1) * 128],
                                 rhs=wc[:], start=True, stop=True)

            # gather per-slot source rows
            tbig = gpool.tile([128, NSL * 80], F32)
            for k in range(NSL):
                nc.gpsimd.indirect_dma_start(
                    out=tbig[:, k * 80:(k + 1) * 80], out_offset=None,
                    in_=T[:],
                    in_offset=bass.IndirectOffsetOnAxis(
                        ap=srcx[:, k:k + 1], axis=0))
            tb4 = tbig[:].rearrange("p (v s c) -> p v s c", v=NVT, s=SLOTS)

            # score = svse + su_src ; w = exp(lrelu(score))
            score = bpool.tile([128, NVT * 64], F32)
            nc.vector.tensor_tensor(
                out=score[:].rearrange("p (v s h) -> p v s h", v=NVT, s=SLOTS),
                in0=sps[:].rearrange("p (v s h) -> p v s h", v=NVT, s=SLOTS),
                in1=tb4[:, :, :, 64:72],
                op=mybir.AluOpType.add)
            # leaky_relu(x, 0.2) = max(x, 0.2x). HW Lrelu has a fixed 0.01
            # slope (ignores alpha), so build it from a scaled copy + max.
            s02 = bpool.tile([128, NVT * 64], F32)
            nc.scalar.mul(s02[:], score[:], 0.2)
            lr_t = bpool.tile([128, NVT * 64], F32)
            nc.vector.tensor_tensor(out=lr_t[:], in0=score[:], in1=s02[:],
                                    op=mybir.AluOpType.max)
            wbig = bpool.tile([128, NVT * 64], F32)
            nc.scalar.activation(wbig[:], lr_t[:],
                                 mybir.ActivationFunctionType.Exp)

            # messages: msg[v,f'] = sum_s w[v,s,H] * h'[v,s,f']
            aggps = bpsum.tile([72, CN], F32, space="PSUM")
            for v in range(NVT):
                m_t = gpool.tile([128, 512], F32)
                nc.vector.tensor_tensor(
                    out=m_t[:].rearrange("p (s h d) -> p s h d", s=SLOTS, h=8),
                    in0=tb4[:, v, :, 0:64].rearrange(
                        "p s (h d) -> p s h d", h=8),
                    in1=wbig[:, v * 64:(v + 1) * 64].rearrange(
                        "p (s h) -> p s h", s=SLOTS).unsqueeze(3).to_broadcast(
                        [128, SLOTS, 8, 8]),
                    op=mybir.AluOpType.mult)
                msgden = bpool.tile([128, 72], F32)
                nc.vector.tensor_reduce(
                    out=msgden[:, 0:64],
                    in_=m_t[:].rearrange("p (s f) -> p f s", s=SLOTS),
                    axis=mybir.AxisListType.X, op=mybir.AluOpType.add)
                nc.vector.tensor_reduce(
                    out=msgden[:, 64:72],
                    in_=wbig[:, v * 64:(v + 1) * 64].rearrange(
                        "p (s h) -> p h s", s=SLOTS),
                    axis=mybir.AxisListType.X, op=mybir.AluOpType.add)
                ohvd = bpool.tile([128, CN], F32)
                nc.vector.tensor_tensor(
                    out=ohvd[:], in0=iotar[:, 0:CN],
                    in1=dlc[:, v:v + 1].to_broadcast([128, CN]),
                    op=mybir.AluOpType.is_equal)
                nc.tensor.matmul(aggps[:], lhsT=msgden[:], rhs=ohvd[:],
                                 start=(v == 0), stop=(v == NVT - 1))

            # normalize: out[f',d] = msg[f',d] * (1/den[H(f'),d])
            # (clamp: pad node columns have den==0; msg there is 0 too)
            denc = bpool.tile([8, CN], F32)
            nc.vector.tensor_scalar_max(out=denc[:], in0=aggps[64:72, :],
                                        scalar1=1e-30)
            recip = bpool.tile([8, CN], F32)
            nc.vector.reciprocal(recip[:], denc[:])
            exps = bpsum.tile([64, CN], F32, space="PSUM")
            nc.tensor.matmul(exps[:], lhsT=e8_sb[:], rhs=recip[:],
                             start=True, stop=True)
            exsb = bpool.tile([64, CN], F32)
            nc.scalar.copy(exsb[:], exps[:])
            outT = bpool.tile([64, CN], F32)
            nc.vector.tensor_tensor(out=outT[:], in0=aggps[0:64, :],
                                    in1=exsb[:], op=mybir.AluOpType.mult)

            # FF: gelu(outT.T @ wff1 + b) @ wff2 + b
            offps = bpsum.tile([CN, 64], F32, space="PSUM")
            for hh in range(2):
                f1 = bpsum.tile([128, CN], F32, space="PSUM",
                                tag=f"ff1_{hh}")
                nc.tensor.matmul(f1[:], lhsT=wff1_sb[:, hh * 128:(hh + 1) * 128],
                                 rhs=outT[:], start=True, stop=True)
                gel = bpool.tile([128, CN], F32, tag=f"gel_{hh}")
                nc.scalar.activation(gel[:], f1[:],
                                     mybir.ActivationFunctionType.Gelu,
                                     bias=bff1_sb[:, hh:hh + 1], scale=1.0)
                nc.tensor.matmul(offps[:], lhsT=gel[:],
                                 rhs=wff2_sb[:, hh * 64:(hh + 1) * 64],
                                 start=(hh == 0), stop=(hh == 1))
            osb = bpool.tile([CN, 64], F32)
            nc.vector.tensor_tensor(out=osb[:], in0=offps[:],
                                    in1=bff2_sb[0:CN, :],
                                    op=mybir.AluOpType.add)
            nc.gpsimd.indirect_dma_start(
                out=out_d[:], out_offset=bass.IndirectOffsetOnAxis(
                    ap=sidx[0:CN, 0:1], axis=0),
                in_=osb[:], in_offset=None)

    nc.compile()
    return nc


# --------------------------------------------------------------------------
# Entry point
# --------------------------------------------------------------------------

_PROG_CACHE = {}


def kernel(**inputs) -> np.ndarray:
    cfg = CFG
    N, NCORES = cfg["N"], cfg["NCORES"]
    wd = _prep_weights(
        cfg,
        np.asarray(inputs["w_in"], np.float32),
        np.asarray(inputs["b_in"], np.float32),
        np.asarray(inputs["w_edge"], np.float32),
        np.asarray(inputs["b_edge"], np.float32),
        np.asarray(inputs["w_att_u"], np.float32),
        np.asarray(inputs["b_att_u"], np.float32),
        np.asarray(inputs["w_att_v"], np.float32),
        np.asarray(inputs["w_att_e"], np.float32),
        np.asarray(inputs["b_att_e"], np.float32),
        np.asarray(inputs["w_ff1"], np.float32),
        np.asarray(inputs["b_ff1"], np.float32),
        np.asarray(inputs["w_ff2"], np.float32),
        np.asarray(inputs["b_ff2"], np.float32),
    )
    per_core, xT_aug, aux = _prep_graph(
        cfg, inputs["x"], inputs["edge_feat"], inputs["src"], inputs["dst"])
    nchunk, n_pad, npc = aux["nchunk"], aux["N_PAD"], aux["NPC"]

    key = (nchunk, n_pad)
    if key not in _PROG_CACHE:
        _PROG_CACHE[key] = _build_program(cfg, nchunk, n_pad, npc)
    nc = _PROG_CACHE[key]

    in_maps = []
    for c in range(NCORES):
        pc = per_core[c]
        in_maps.append(dict(
            xT=xT_aug, waug=wd["waug"], wsd=wd["wsd"],
            beff_bc=wd["beff_bc"], e8=wd["e8"], wff1=wd["wff1"],
            wff2=wd["wff2"], bff1_cols=wd["bff1_cols"], bff2_bc=wd["bff2_bc"],
            A_gather=pc["A_gather"], A_scatter=pc["A_scatter"],
            A_src=pc["A_src"], A_dlc=pc["A_dlc"], A_dlr=pc["A_dlr"],
            A_ef=pc["A_ef"],
        ))

    res = run_bass_kernel_spmd(nc, in_maps, core_ids=list(range(NCORES)),
                               trace=bool(int(os.environ.get("DGAT_TRACE", "0"))))
    out = np.empty((N, 64), np.float32)
    for c in range(NCORES):
        out[c * npc:(c + 1) * npc] = res.results[c]["out"][:npc]
    # undo the head-major feature permutation: out'[:, f'] corresponds to
    # original feature f = fmap[f']  -> already handled? No: FF consumed the
    # permutation internally (wff1 rows permuted), so device output is in
    # ORIGINAL feature order. Nothing to undo.
    globals()["_LAST_RESULTS"] = res
    return out


# revision 27
# speedup vs baseline: 3.9314x; 3.9314x over previous
"""DGAT (edge-featured multi-head GAT layer + FF) Trainium2 Bass kernel.

Strategy (graph/data parallel over 8 NeuronCores):
  - Nodes are partitioned into 8 contiguous destination-node shards; each core
    owns the edges whose dst falls in its shard (host sorts edges by dst).
  - Each core builds a node table T[n] = [h'(64, head-major) | su(8) | sv(8)]
    for ALL nodes (replicated compute; h = x@w_in+b etc. folded into one
    matmul with an augmented-ones row for biases).
  - Edges are packed into "virtual rows" of 8 slots each (vrow = up to 8 edges
    sharing one dst). Chunks of <=112 dst nodes own <=256 vrows (2 vtiles of
    128). Per-edge source features arrive via indirect-DMA gathers of T rows.
  - Scores: se+sv come from one matmul with a combined stationary
    [one-hot(dst) | per-slot edge features] against [sv+b_eff | w_eff diag];
    su comes from the gathered T rows. LeakyReLU+exp on the scalar engine.
  - Weighted messages + softmax denominators are segment-summed over each
    chunk via a one-hot matmul into PSUM, normalized, then the FF block
    (gelu MLP) runs per chunk fully on-chip. Results scatter back by node id.
"""

import math
import os
from contextlib import ExitStack

import numpy as np

import concourse.bass as bass
import concourse.tile as tile
from concourse import bacc, mybir
from concourse.bass_utils import run_bass_kernel_spmd

F32 = mybir.dt.float32
F16 = mybir.dt.float16
I32 = mybir.dt.int32

# Problem constants (hardcoded per contract).
CFG = dict(
    N=100000,
    E=1600000,
    DIM=64,
    H=8,
    HD=8,
    HIDDEN=256,
    NCORES=8,
    SLOTS=8,          # edge slots per virtual row
    NVT=2,            # vtiles (of 128 vrows) per chunk
    CN_MAX=112,       # max dst nodes per chunk (112 + 16 ef rows = 128)
    TRASH_SU=-80.0,   # su value of the trash table row: exp(lrelu(-80)) ~ 1e-7
)


# --------------------------------------------------------------------------
# Host-side preprocessing
# --------------------------------------------------------------------------

def _fmap(cfg):
    """f' (head-major: H*HD+hd) -> original feature index f = hd*H + H_idx."""
    H, HD = cfg["H"], cfg["HD"]
    fp = np.arange(H * HD)
    Hi, hd = fp // HD, fp % HD
    return (hd * H + Hi).astype(np.int64)


def _prep_weights(cfg, w_in, b_in, w_edge, b_edge, w_att_u, b_att_u, w_att_v,
                  w_att_e, b_att_e, w_ff1, b_ff1, w_ff2, b_ff2):
    H, HD, DIM, SLOTS = cfg["H"], cfg["HD"], cfg["DIM"], cfg["SLOTS"]
    fm = _fmap(cfg)
    w_in_p = w_in[:, fm]                       # columns permuted to head-major
    b_in_p = b_in[fm]
    w_uv = np.concatenate([w_att_u, w_att_v], axis=1)      # [64,16]
    w_uvfold = w_in @ w_uv                                  # [64,16]
    b_su = b_in @ w_att_u + b_att_u
    b_sv = b_in @ w_att_v
    # rows: 0:64 weights, 64 bias (ones row), 65 "is_trash" indicator row
    waug = np.zeros((66, 80), np.float32)
    waug[:64, 0:64] = w_in_p
    waug[:64, 64:80] = w_uvfold
    waug[64, 0:64] = b_in_p
    waug[64, 64:72] = b_su
    waug[64, 72:80] = b_sv
    waug[65, 64:72] = cfg["TRASH_SU"]

    w_eff = w_edge @ w_att_e                                # [2,8]
    b_eff = b_edge @ w_att_e + b_att_e                      # [8]
    # W rows 112:128: row 2*s+j, col s2*8+Hh = (s==s2) * w_eff[j,Hh]
    wsd = np.zeros((16, 64), np.float32)
    for s in range(SLOTS):
        wsd[2 * s, s * 8:(s + 1) * 8] = w_eff[0]
        wsd[2 * s + 1, s * 8:(s + 1) * 8] = w_eff[1]
    beff_bc = np.tile(b_eff[None, :], (128, SLOTS)).astype(np.float32)  # [128,64]

    e8 = np.zeros((8, 64), np.float32)
    for Hh in range(H):
        e8[Hh, Hh * 8:(Hh + 1) * 8] = 1.0

    wff1_p = w_ff1[fm, :]                                   # rows permuted
    bff1_cols = b_ff1.reshape(2, 128).T.astype(np.float32).copy()  # [128,2]
    bff2_bc = np.tile(b_ff2[None, :], (128, 1)).astype(np.float32)  # [128,64]
    return dict(
        waug=waug, wsd=wsd, beff_bc=beff_bc, e8=e8,
        wff1=np.ascontiguousarray(wff1_p, np.float32),
        wff2=np.ascontiguousarray(w_ff2, np.float32),
        bff1_cols=bff1_cols, bff2_bc=bff2_bc,
    )


def _prep_graph(cfg, x, edge_feat, src, dst):
    """Build per-core chunk/vrow arrays. Returns (per_core list, nchunk, aux)."""
    N, E, NCORES = cfg["N"], cfg["E"], cfg["NCORES"]
    SLOTS, NVT, CN_MAX = cfg["SLOTS"], cfg["NVT"], cfg["CN_MAX"]
    VPC = NVT * 128                       # vrows per chunk
    NPC = N // NCORES                     # nodes per core
    N_PAD = ((N + 511) // 512) * 512      # phase-A padded rows
    if N_PAD == N:
        N_PAD += 512                      # need at least one trash column
    TRASH = N                             # trash row index in T (a pad row)

    src = np.asarray(src).astype(np.int64).ravel()
    dst = np.asarray(dst).astype(np.int64).ravel()
    ef = np.asarray(edge_feat, np.float32).reshape(E, 2)

    order = np.argsort(dst, kind="stable")
    dst_s = dst[order]
    src_s = src[order]
    ef_s = ef[order]

    deg = np.bincount(dst_s, minlength=N)
    node_start = np.concatenate([[0], np.cumsum(deg)])     # [N+1]
    nvr = np.maximum(1, (deg + SLOTS - 1) // SLOTS)        # vrows per node
    # edge -> (vrow_within_node, slot)
    erank = np.arange(E) - node_start[dst_s]               # rank within node
    e_vr_in_node = erank // SLOTS
    e_slot = erank % SLOTS

    # Greedy chunking per core: <=CN_MAX nodes and <=VPC vrows per chunk.
    chunks_per_core = []       # list of list of (n0, n1) node ranges (global)
    for c in range(NCORES):
        lo, hi = c * NPC, (c + 1) * NPC
        ch = []
        n0 = lo
        while n0 < hi:
            n1 = n0
            vsum = 0
            while n1 < hi and (n1 - n0) < CN_MAX and vsum + nvr[n1] <= VPC:
                vsum += nvr[n1]
                n1 += 1
            assert n1 > n0, f"node {n0} has nvr {nvr[n0]} > {VPC}"
            ch.append((n0, n1))
            n0 = n1
        chunks_per_core.append(ch)
    nchunk = max(len(ch) for ch in chunks_per_core)

    per_core = []
    for c in range(NCORES):
        lo = c * NPC
        ch = chunks_per_core[c]
        A_scatter = np.full((nchunk, 128), NPC, np.int32)
        A_gnodes = np.full((nchunk, 128), N, np.int64)
        A_src = np.full((nchunk, NVT * SLOTS, 128), N, np.int64)
        A_dlc = np.full((nchunk, 128, NVT), -1.0, np.float16)
        A_dlr = np.full((nchunk, NVT * 128), -1.0, np.float16)
        A_ef = np.zeros((nchunk, 2 * SLOTS, NVT * 128), np.float16)
        for k, (n0, n1) in enumerate(ch):
            cn = n1 - n0
            nodes = np.arange(n0, n1)
            A_gnodes[k, :cn] = nodes
            A_scatter[k, :cn] = nodes - lo
            # vrows of this chunk: node d contributes nvr[d] vrows
            vr_nodes = np.repeat(nodes, nvr[n0:n1])        # [<=VPC] global ids
            nv = len(vr_nodes)
            dloc = (vr_nodes - n0).astype(np.float16)
            A_dlr[k, :nv] = dloc
            # vrow index layout: flat v in [0, VPC) -> (vtile v//128, row v%128)
            vt, vp = np.arange(nv) // 128, np.arange(nv) % 128
            A_dlc[k, vp, vt] = dloc
            # edges of this chunk
            e0, e1 = node_start[n0], node_start[n1]
            if e1 > e0:
                ee = np.arange(e0, e1)
                # flat vrow id within chunk
                vr_base = np.concatenate([[0], np.cumsum(nvr[n0:n1])])
                evr = vr_base[dst_s[ee] - n0] + e_vr_in_node[ee]
                es = e_slot[ee]
                evt, evp = evr // 128, evr % 128
                A_src[k, evt * SLOTS + es, evp] = src_s[ee]
                # ef rows: row 2*s+j, col vt*128+vp
                A_ef[k, 2 * es, evt * 128 + evp] = ef_s[ee, 0]
                A_ef[k, 2 * es + 1, evt * 128 + evp] = ef_s[ee, 1]
        per_core.append(dict(
            A_scatter=A_scatter, A_src=A_src, A_gnodes=A_gnodes,
            A_dlc=A_dlc, A_dlr=A_dlr, A_ef=A_ef,
        ))

    x = np.asarray(x, np.float32)
    # augmented per-node input rows [N+1, 66]: x | ones | is_trash
    xa = np.zeros((N + 1, 66), np.float16)
    xa[:N, :64] = x.astype(np.float16)
    xa[:N, 64] = 1.0
    xa[N, 65] = 1.0                       # trash row
    for pc in per_core:
        # [nchunk, 66, NSL*128]: per chunk the gathered slot columns
        pc["A_xg"] = np.ascontiguousarray(
            xa[pc.pop("A_src").reshape(nchunk, -1)].transpose(0, 2, 1))
        pc["A_xc"] = np.ascontiguousarray(
            xa[pc.pop("A_gnodes")].transpose(0, 2, 1))
    aux = dict(N_PAD=N_PAD, NPC=NPC, nchunk=nchunk)
    return per_core, None, aux


# --------------------------------------------------------------------------
# Device program
# --------------------------------------------------------------------------

def _build_program(cfg, nchunk, n_pad, npc):
    SLOTS, NVT, CN = cfg["SLOTS"], cfg["NVT"], cfg["CN_MAX"]
    TRASH_SU = cfg["TRASH_SU"]
    NVROW = NVT * 128
    NSL = NVT * SLOTS

    nc = bacc.Bacc("TRN2", target_bir_lowering=False, debug=False)

    def din(name, shape, dt=F32):
        return nc.dram_tensor(name, list(shape), dt, kind="ExternalInput").ap()

    waug_d = din("waug", (66, 80), F16)
    wsd_d = din("wsd", (16, 64), F16)
    beff_d = din("beff_bc", (128, 64), F16)
    e8_d = din("e8", (8, 64))
    wff1_d = din("wff1", (64, 256), F16)
    wff2_d = din("wff2", (256, 64), F16)
    bff1_d = din("bff1_cols", (128, 2))
    bff2_d = din("bff2_bc", (128, 64))
    as_d = din("A_scatter", (nchunk, 128), I32)
    axg_d = din("A_xg", (nchunk, 66, NSL * 128), F16)
    axc_d = din("A_xc", (nchunk, 66, 128), F16)
    adlc_d = din("A_dlc", (nchunk, 128, NVT), F16)
    adlr_d = din("A_dlr", (nchunk, NVROW), F16)
    aef_d = din("A_ef", (nchunk, 2 * SLOTS, NVROW), F16)

    out_d = nc.dram_tensor("out", [npc + 1, 64], F32,
                           kind="ExternalOutput").ap()

    with ExitStack() as ctx:
        tc = ctx.enter_context(tile.TileContext(nc))
        cpool = ctx.enter_context(tc.tile_pool(name="consts", bufs=1))

        # ---- constants in SBUF
        waug_sb = cpool.tile([66, 80], F16)
        nc.sync.dma_start(out=waug_sb[:], in_=waug_d[:])
        beff_sb = cpool.tile([128, 64], F16)
        nc.sync.dma_start(out=beff_sb[:], in_=beff_d[:])
        e8_sb = cpool.tile([8, 64], F32)
        nc.sync.dma_start(out=e8_sb[:], in_=e8_d[:])
        wff1_sb = cpool.tile([64, 256], F16)
        nc.sync.dma_start(out=wff1_sb[:], in_=wff1_d[:])
        wff2_sb = cpool.tile([128, 128], F16)
        # [256,64] -> [128, 2, 64]: row r=half*128+p  maps to [p, half, :]
        nc.sync.dma_start(
            out=wff2_sb[:].rearrange("p (h c) -> p h c", h=2),
            in_=wff2_d[:].rearrange("(h p) c -> p h c", h=2))
        bff1_sb = cpool.tile([128, 2], F32)
        nc.sync.dma_start(out=bff1_sb[:], in_=bff1_d[:])
        bff2_sb = cpool.tile([128, 64], F32)
        nc.sync.dma_start(out=bff2_sb[:], in_=bff2_d[:])

        ones_sb = cpool.tile([1, 128], F16)
        nc.vector.memset(ones_sb[:], 1.0)
        iotac_i = cpool.tile([128, 1], I32)
        nc.gpsimd.iota(iotac_i[:], pattern=[[0, 1]], base=0,
                       channel_multiplier=1)
        iotac = cpool.tile([128, 1], F32)
        nc.vector.tensor_copy(out=iotac[:], in_=iotac_i[:])
        iotar_i = cpool.tile([128, 128], I32)
        nc.gpsimd.iota(iotar_i[:], pattern=[[1, 128]], base=0,
                       channel_multiplier=0)
        iotar = cpool.tile([128, 128], F16)
        nc.vector.tensor_copy(out=iotar[:], in_=iotar_i[:])

        # persistent combined-stationary W tile: rows 0:112 per-chunk sv+beff,
        # rows 112:128 constant w_eff slot-diagonal
        wc = cpool.tile([128, 64], F16)
        nc.sync.dma_start(out=wc[112:128, :], in_=wsd_d[:])

        # ---- Phase B: chunks
        bpool = ctx.enter_context(tc.tile_pool(name="chunk", bufs=3))
        gpool = ctx.enter_context(tc.tile_pool(name="gath", bufs=3))
        bpsum = ctx.enter_context(tc.tile_pool(name="chunk_ps", bufs=1,
                                               space="PSUM"))
        spsum = ctx.enter_context(tc.tile_pool(name="slot_ps", bufs=2,
                                               space="PSUM"))
        sppool = ctx.enter_context(tc.tile_pool(name="score_ps", bufs=2,
                                                space="PSUM"))
        for ck in range(nchunk):
            sidx = bpool.tile([128, 1], I32)
            nc.sync.dma_start(out=sidx[:], in_=as_d[ck, :, None])
            dlc = bpool.tile([128, NVT], F16)
            nc.sync.dma_start(out=dlc[:], in_=adlc_d[ck])
            dlr = bpool.tile([1, NVROW], F16)
            nc.sync.dma_start(out=dlr[:], in_=adlr_d[ck, None, :])

            # chunk-node sv via on-the-fly transform of host-gathered x cols
            xct = bpool.tile([66, 128], F16)
            nc.sync.dma_start(out=xct[:], in_=axc_d[ck])
            tchp = spsum.tile([128, 320], F32, space="PSUM", tag="slotps")
            nc.tensor.matmul(tchp[:, 0:80], lhsT=xct[:], rhs=waug_sb[:],
                             start=True, stop=True)
            tch = bpool.tile([128, 8], F16)
            nc.scalar.copy(tch[:], tchp[:, 72:80])
            nc.vector.tensor_tensor(
                out=wc[0:CN, :].rearrange("p (s c) -> p s c", s=SLOTS),
                in0=tch[0:CN, 0:8].unsqueeze(1).to_broadcast(
                    [CN, SLOTS, 8]),
                in1=beff_sb[0:CN, :].rearrange("p (s c) -> p s c", s=SLOTS),
                op=mybir.AluOpType.add)

            # U = [one-hot(dloc) ; ef rows]
            dbc = bpsum.tile([CN, NVROW], F32, space="PSUM")
            nc.tensor.matmul(dbc[:], lhsT=ones_sb[0:1, 0:CN],
                             rhs=dlr[0:1, :], start=True, stop=True)
            u_t = bpool.tile([128, NVROW], F16)
            nc.sync.dma_start(out=u_t[CN:128, :], in_=aef_d[ck])
            nc.vector.tensor_tensor(
                out=u_t[0:CN, :], in0=dbc[:],
                in1=iotac[0:CN, 0:1].to_broadcast([CN, NVROW]),
                op=mybir.AluOpType.is_equal)

            # sv+se+b_eff per (vrow, slot, head)
            sps = sppool.tile([128, NVT * 64], F32, space="PSUM")
            for v in range(NVT):
                nc.tensor.matmul(sps[:, v * 64:(v + 1) * 64],
                                 lhsT=u_t[:, v * 128:(v + 1) * 128],
                                 rhs=wc[:], start=True, stop=True)

            # per-slot source rows: transform host-gathered x columns
            # (dense DMA + matmul instead of 16 indirect gathers)
            xgt = gpool.tile([66, NSL * 128], F16)
            nc.sync.dma_start(out=xgt[:], in_=axg_d[ck])
            tbig = gpool.tile([128, NSL * 80], F16)
            for g in range(4):
                sp = spsum.tile([128, 320], F32, space="PSUM", tag="slotps")
                for j in range(4):
                    k = g * 4 + j
                    nc.tensor.matmul(sp[:, j * 80:(j + 1) * 80],
                                     lhsT=xgt[:, k * 128:(k + 1) * 128],
                                     rhs=waug_sb[:], start=True, stop=True)
                nc.scalar.copy(tbig[:, g * 320:(g + 1) * 320], sp[:])
            tb4 = tbig[:].rearrange("p (v s c) -> p v s c", v=NVT, s=SLOTS)

            # score = svse + su_src ; w = exp(lrelu(score))
            score = bpool.tile([128, NVT * 64], F32)
            nc.vector.tensor_tensor(
                out=score[:].rearrange("p (v s h) -> p v s h", v=NVT, s=SLOTS),
                in0=sps[:].rearrange("p (v s h) -> p v s h", v=NVT, s=SLOTS),
                in1=tb4[:, :, :, 64:72],
                op=mybir.AluOpType.add)
            # leaky_relu(x, 0.2) = max(x, 0.2x). HW Lrelu has a fixed 0.01
            # slope (ignores alpha), so build it from a scaled copy + max.
            s02 = bpool.tile([128, NVT * 64], F32)
            nc.scalar.mul(s02[:], score[:], 0.2)
            lr_t = bpool.tile([128, NVT * 64], F32)
            nc.vector.tensor_tensor(out=lr_t[:], in0=score[:], in1=s02[:],
                                    op=mybir.AluOpType.max)
            wbig = bpool.tile([128, NVT * 64], F16)
            nc.scalar.activation(wbig[:], lr_t[:],
                                 mybir.ActivationFunctionType.Exp)

            # messages: msg[v,f'] = sum_s w[v,s,H] * h'[v,s,f']
            aggps = bpsum.tile([72, CN], F32, space="PSUM")
            for v in range(NVT):
                m_t = gpool.tile([128, 512], F16)
                nc.vector.tensor_tensor(
                    out=m_t[:].rearrange("p (s h d) -> p s h d", s=SLOTS, h=8),
                    in0=tb4[:, v, :, 0:64].rearrange(
                        "p s (h d) -> p s h d", h=8),
                    in1=wbig[:, v * 64:(v + 1) * 64].rearrange(
                        "p (s h) -> p s h", s=SLOTS).unsqueeze(3).to_broadcast(
                        [128, SLOTS, 8, 8]),
                    op=mybir.AluOpType.mult)
                msgden = bpool.tile([128, 72], F16)
                with nc.allow_low_precision("fp16 slot sums, fp32 psum agg"):
                    nc.vector.tensor_reduce(
                        out=msgden[:, 0:64],
                        in_=m_t[:].rearrange("p (s f) -> p f s", s=SLOTS),
                        axis=mybir.AxisListType.X, op=mybir.AluOpType.add)
                    nc.vector.tensor_reduce(
                        out=msgden[:, 64:72],
                        in_=wbig[:, v * 64:(v + 1) * 64].rearrange(
                            "p (s h) -> p h s", s=SLOTS),
                        axis=mybir.AxisListType.X, op=mybir.AluOpType.add)
                ohvd = bpool.tile([128, CN], F16)
                nc.vector.tensor_tensor(
                    out=ohvd[:], in0=iotar[:, 0:CN],
                    in1=dlc[:, v:v + 1].to_broadcast([128, CN]),
                    op=mybir.AluOpType.is_equal)
                nc.tensor.matmul(aggps[:], lhsT=msgden[:], rhs=ohvd[:],
                                 start=(v == 0), stop=(v == NVT - 1))

            # normalize: out[f',d] = msg[f',d] * (1/den[H(f'),d])
            # (clamp: pad node columns have den==0; msg there is 0 too)
            denc = bpool.tile([8, CN], F32)
            nc.vector.tensor_scalar_max(out=denc[:], in0=aggps[64:72, :],
                                        scalar1=1e-30)
            recip = bpool.tile([8, CN], F32)
            nc.vector.reciprocal_approx_fast(out=recip[:], in_=denc[:])
            fex = bpsum.tile([128, 2 * CN + 112], F32, space="PSUM",
                             tag="fex")
            exps = fex[0:64, 2 * CN:2 * CN + 112]
            nc.tensor.matmul(exps, lhsT=e8_sb[:], rhs=recip[:],
                             start=True, stop=True)
            exsb = bpool.tile([64, CN], F32)
            nc.scalar.copy(exsb[:], exps)
            outT = bpool.tile([64, CN], F16)
            nc.vector.tensor_tensor(out=outT[:], in0=aggps[0:64, :],
                                    in1=exsb[:], op=mybir.AluOpType.mult)

            # FF: gelu(outT.T @ wff1 + b) @ wff2 + b
            offps = bpsum.tile([CN, 64], F32, space="PSUM")
            for hh in range(2):
                f1 = fex[:, hh * CN:(hh + 1) * CN]
                nc.tensor.matmul(f1, lhsT=wff1_sb[:, hh * 128:(hh + 1) * 128],
                                 rhs=outT[:], start=True, stop=True)
                gel = bpool.tile([128, CN], F16, tag=f"gel_{hh}")
                nc.scalar.activation(gel[:], f1,
                                     mybir.ActivationFunctionType.Gelu,
                                     bias=bff1_sb[:, hh:hh + 1], scale=1.0)
                nc.tensor.matmul(offps[:], lhsT=gel[:],
                                 rhs=wff2_sb[:, hh * 64:(hh + 1) * 64],
                                 start=(hh == 0), stop=(hh == 1))
            osb = bpool.tile([CN, 64], F32)
            nc.vector.tensor_tensor(out=osb[:], in0=offps[:],
                                    in1=bff2_sb[0:CN, :],
                                    op=mybir.AluOpType.add)
            nc.gpsimd.indirect_dma_start(
                out=out_d[:], out_offset=bass.IndirectOffsetOnAxis(
                    ap=sidx[0:CN, 0:1], axis=0),
                in_=osb[:], in_offset=None)

    nc.compile()
    return nc


# --------------------------------------------------------------------------
# Entry point
# --------------------------------------------------------------------------

_PROG_CACHE = {}


def kernel(**inputs) -> np.ndarray:
    cfg = CFG
    N, NCORES = cfg["N"], cfg["NCORES"]
    wd = _prep_weights(
        cfg,
        np.asarray(inputs["w_in"], np.float32),
        np.asarray(inputs["b_in"], np.float32),
        np.asarray(inputs["w_edge"], np.float32),
        np.asarray(inputs["b_edge"], np.float32),
        np.asarray(inputs["w_att_u"], np.float32),
        np.asarray(inputs["b_att_u"], np.float32),
        np.asarray(inputs["w_att_v"], np.float32),
        np.asarray(inputs["w_att_e"], np.float32),
        np.asarray(inputs["b_att_e"], np.float32),
        np.asarray(inputs["w_ff1"], np.float32),
        np.asarray(inputs["b_ff1"], np.float32),
        np.asarray(inputs["w_ff2"], np.float32),
        np.asarray(inputs["b_ff2"], np.float32),
    )
    per_core, xT_aug, aux = _prep_graph(
        cfg, inputs["x"], inputs["edge_feat"], inputs["src"], inputs["dst"])
    nchunk, n_pad, npc = aux["nchunk"], aux["N_PAD"], aux["NPC"]

    key = (nchunk, n_pad)
    if key not in _PROG_CACHE:
        _PROG_CACHE[key] = _build_program(cfg, nchunk, n_pad, npc)
    nc = _PROG_CACHE[key]

    in_maps = []
    for c in range(NCORES):
        pc = per_core[c]
        in_maps.append(dict(
            waug=wd["waug"].astype(np.float16), wsd=wd["wsd"].astype(np.float16),
            beff_bc=wd["beff_bc"].astype(np.float16), e8=wd["e8"],
            wff1=wd["wff1"].astype(np.float16),
            wff2=wd["wff2"].astype(np.float16),
            bff1_cols=wd["bff1_cols"], bff2_bc=wd["bff2_bc"],
            A_scatter=pc["A_scatter"], A_xg=pc["A_xg"], A_xc=pc["A_xc"],
            A_dlc=pc["A_dlc"], A_dlr=pc["A_dlr"], A_ef=pc["A_ef"],
        ))

    res = run_bass_kernel_spmd(nc, in_maps, core_ids=list(range(NCORES)),
                               trace=bool(int(os.environ.get("DGAT_TRACE", "0"))))
    out = np.empty((N, 64), np.float32)
    for c in range(NCORES):
        out[c * npc:(c + 1) * npc] = res.results[c]["out"][:npc]
    # undo the head-major feature permutation: out'[:, f'] corresponds to
    # original feature f = fmap[f']  -> already handled? No: FF consumed the
    # permutation internally (wff1 rows permuted), so device output is in
    # ORIGINAL feature order. Nothing to undo.
    globals()["_LAST_RESULTS"] = res
    return out
